# revision 26
# baseline (speedup 1.0000x reference)
"""LinearAttention Trainium2 Bass kernel.

kernel(**inputs) takes the full unsharded inputs from setup_inputs() and
returns the full output. Shards data-parallel over batch (b=8) across 8
NeuronCores; each core computes one batch item:

  qkv = w_qkv @ x            (layout B on chip: [n, 768], n on partitions)
  q = softmax_d(q); k = softmax_n(k)
  ctx[h] = ek_h^T @ v_h      (accumulated over n in PSUM; per-pair rhs is
                              [v_pair | ones] so each ctx matmul streams only
                              129 columns; the ones column yields sum_n ek)
  M^T = blockdiag(ctx/s_k)^T @ w_out^T   (folded once between passes)
  out = M @ eqnA + b_out     (eqnA = PE-transposed normalized exp(q))

The pass-1 loop is software-pipelined: ctx matmuls lag the producing
iteration by 1 and the q transposes / eqnA copies by LAG so the tensor
queue never waits on the scalar/vector/gpsimd chains.

Matmul operands are bf16; accumulation stays fp32 in PSUM. Output is
stored bf16 and widened to fp32 on the host.
"""

import numpy as np
import ml_dtypes

import concourse.bass as bass
import concourse.tile as tile
from concourse import bacc, mybir
from concourse.bass_utils import run_bass_kernel_spmd
from concourse.masks import make_identity

F32 = mybir.dt.float32
BF16 = mybir.dt.bfloat16
AF = mybir.ActivationFunctionType
X_AXIS = mybir.AxisListType.X

C = 128
N = 16384
HEADS = 4
DH = 64
INNER = HEADS * DH          # 256
QKV = 3 * INNER             # 768
NB = 512
SUB = NB // 128
NBLK = N // NB              # 32
NSUB = N // 128             # 128
LAG = 4                     # transpose/eqnA-copy skew (iterations)


def build_nc():
    nc = bacc.Bacc("TRN2", target_bir_lowering=False, debug=False, num_devices=8)

    x = nc.dram_tensor("x", [C, N], BF16, kind="ExternalInput")
    wqT = nc.dram_tensor("wqT", [C, QKV], BF16, kind="ExternalInput")
    woT = nc.dram_tensor("woT", [INNER, C], BF16, kind="ExternalInput")
    bo = nc.dram_tensor("bo", [C, 1], F32, kind="ExternalInput")
    out = nc.dram_tensor("out", [C, N], BF16, kind="ExternalOutput")

    with tile.TileContext(nc) as tc:
        with (
            tc.tile_pool(name="consts", bufs=1) as consts,
            tc.tile_pool(name="eqa", bufs=1) as eqa,
            tc.tile_pool(name="xin", bufs=4) as xin,
            tc.tile_pool(name="work", bufs=5) as work,
            tc.tile_pool(name="eqn", bufs=6) as eqnp,
            tc.tile_pool(name="vtp", bufs=5) as vtp,
            tc.tile_pool(name="small", bufs=4) as small,
        ):
            wq_s = consts.tile([C, QKV], BF16)
            nc.sync.dma_start(out=wq_s, in_=wqT[:, :])
            wo_s = consts.tile([C, 2, C], BF16)
            nc.sync.dma_start(out=wo_s[:, 0, :], in_=woT[0:128, :])
            nc.sync.dma_start(out=wo_s[:, 1, :], in_=woT[128:256, :])
            bo_s = consts.tile([C, 1], F32)
            nc.sync.dma_start(out=bo_s, in_=bo[:, :])
            ident = consts.tile([C, C], BF16)
            make_identity(nc, ident)

            # layout-A normalized exp(q): [:, 0, :] = heads 0/1, [:, 1, :] = 2/3
            eqnA = eqa.tile([C, 2, N], BF16)
            MT01 = consts.tile([C, C], BF16)
            MT23 = consts.tile([C, C], BF16)
            # blockdiag scratch: zeroed once here, only diag blocks written
            # at fold time
            bd01 = consts.tile([C, C], BF16)
            bd23 = consts.tile([C, C], BF16)
            nc.gpsimd.memset(bd01, 0.0)
            nc.gpsimd.memset(bd23, 0.0)

            with (
                tc.tile_pool(name="qkvp", bufs=2, space="PSUM") as qkvp,
                tc.tile_pool(name="trp", bufs=1, space="PSUM") as trp,
                tc.tile_pool(name="ctxp", bufs=1, space="PSUM") as ctxp,
            ):
                # both ctx accumulators in one PSUM bank: [:, 0, :] = heads
                # 0/1, [:, 1, :] = heads 2/3; col 128 = sum_n ek (ones trick).
                # Stride 160 keeps the second matmul's output 64B-aligned.
                ctx_pad = ctxp.tile([C, 2, 160], F32)
                ctx = ctx_pad[:, :, 0:129]
                # 3 transpose slots share one PSUM bank (512B each),
                # manually rotated; writers tensor-only, readers vector-only
                tr_all = trp.tile([C, 3, 2, C], BF16)

                x_blk = None
                qkv_t = [None] * NSUB   # PSUM qkv tiles
                eqk_t = [None] * NSUB   # SBUF exp(q|k)
                vt_t = [None] * NSUB    # SBUF [v01|ones|v23|ones]
                eqn_t = [None] * NSUB   # SBUF normalized exp(q)
                tr_t = [None] * NSUB    # PSUM transposed eqn

                for it in range(NSUB + LAG):
                    t = it if it < NSUB else None
                    if t is not None:
                        blk, s = divmod(t, SUB)
                        if s == 0:
                            x_blk = xin.tile([C, NB], BF16, tag="x_blk")
                            nc.sync.dma_start(
                                out=x_blk, in_=x[:, blk * NB : (blk + 1) * NB]
                            )
                        xs = x_blk[:, s * 128 : (s + 1) * 128]

                        # paired qkv PSUM tile (3 banks): [qk_e|qk_o|v_e v_o]
                        if t % 2 == 0:
                            qkv2 = qkvp.tile([C, 1536], F32, tag="qkv")
                        sub = t % 2
                        nc.tensor.matmul(
                            qkv2[:, sub * 512 : sub * 512 + 512],
                            lhsT=xs, rhs=wq_s[:, 0:512],
                            start=True, stop=True, skip_group_check=True,
                        )
                        nc.tensor.matmul(
                            qkv2[:, 1024 + sub * 256 : 1280 + sub * 256],
                            lhsT=xs, rhs=wq_s[:, 512:768],
                            start=True, stop=True, skip_group_check=True,
                        )

                    # skewed tensor work: transposes for t-LAG
                    tp = it - LAG
                    if 0 <= tp < NSUB:
                        tr = tr_all[:, tp % 3]
                        tr_t[tp] = tr
                        eqns = eqn_t[tp]
                        nc.tensor.transpose(tr[:, 0, :], eqns[:, 0:2, :], ident)
                        nc.tensor.transpose(tr[:, 1, :], eqns[:, 2:4, :], ident)
                        eqn_t[tp] = None

                    # skewed ctx accumulation for t-3: the pair-batched
                    # exp retires up to ~2.3 iters after the even
                    # sub-iteration, so lag 3 keeps the gate pre-satisfied
                    tc_ = it - 3
                    if 0 <= tc_ < NSUB:
                        eqkc = eqk_t[tc_]
                        vtc = vt_t[tc_]
                        nc.tensor.matmul(
                            ctx[:, 0, :], lhsT=eqkc[:, 4:6, :], rhs=vtc[:, 0, :],
                            start=(tc_ == 0), stop=(tc_ == NSUB - 1),
                            skip_group_check=True,
                        )
                        nc.tensor.matmul(
                            ctx[:, 1, :], lhsT=eqkc[:, 6:8, :], rhs=vtc[:, 1, :],
                            start=(tc_ == 0), stop=(tc_ == NSUB - 1),
                            skip_group_check=True,
                        )
                        if tc_ >= 2:
                            eqk_t[tc_ - 1] = None
                            vt_t[tc_ - 1] = None

                    if t is not None:
                        # v copy stays per-iteration (keeps the ctx gate
                        # fine-grained); exp is pair-batched below
                        vt = vtp.tile([C, 2, 129], BF16, tag="vt")
                        vt_t[t] = vt
                        nc.scalar.copy(
                            vt[:, :, 0:128],
                            qkv2[:, 1024 + sub * 256 : 1280 + sub * 256],
                        )
                        nc.gpsimd.memset(vt[:, :, 128:129], 1.0)

                    if t is not None and t % 2 == 1:
                        # one exp over both sub-iterations' q|k halves
                        eqk2 = work.tile([C, 2, 8, DH], BF16, tag="eqk")
                        eqk_t[t - 1] = eqk2[:, 0]
                        eqk_t[t] = eqk2[:, 1]
                        nc.scalar.activation(
                            eqk2[:, :, :, :], qkv2[:, 0:1024], AF.Exp
                        )
                        for ts in (t - 1, t):
                            eqkv = eqk_t[ts]
                            sq = small.tile([C, HEADS, 1], F32, tag="sq")
                            nc.vector.reduce_sum(
                                sq, eqkv[:, 0:4, :], axis=X_AXIS
                            )
                            rq = small.tile([C, HEADS, 1], F32, tag="rq")
                            nc.vector.reciprocal(rq, sq)
                            eqn = eqnp.tile([C, HEADS, DH], BF16, tag="eqn")
                            eqn_t[ts] = eqn
                            nc.gpsimd.tensor_mul(
                                eqn, eqkv[:, 0:4, :],
                                rq.broadcast_to([C, HEADS, DH]),
                            )

                    # skewed eqnA copy for t-LAG (after its transposes above)
                    if 0 <= tp < NSUB:
                        nc.vector.tensor_copy(
                            eqnA[:, :, tp * 128 : (tp + 1) * 128], tr_t[tp]
                        )
                        tr_t[tp] = None

                # ---- fold: MT = (blockdiag(ctx/s_k))^T @ w_out^T ----
                r01 = small.tile([C, 1], F32, tag="r01")
                r23 = small.tile([C, 1], F32, tag="r23")
                nc.vector.reciprocal(r01, ctx[:, 0, 128:129])
                nc.vector.reciprocal(r23, ctx[:, 1, 128:129])
                nc.vector.tensor_scalar_mul(
                    bd01[0:64, 0:64], ctx[0:64, 0, 0:64], r01[0:64, 0:1]
                )
                nc.vector.tensor_scalar_mul(
                    bd01[64:128, 64:128], ctx[64:128, 0, 64:128], r01[64:128, 0:1]
                )
                nc.vector.tensor_scalar_mul(
                    bd23[0:64, 0:64], ctx[0:64, 1, 0:64], r23[0:64, 0:1]
                )
                nc.vector.tensor_scalar_mul(
                    bd23[64:128, 64:128], ctx[64:128, 1, 64:128], r23[64:128, 0:1]
                )

                for pair, bd, mt in ((0, bd01, MT01), (1, bd23, MT23)):
                    tb = tr_all[:, pair]
                    nc.tensor.transpose(tb[:, 0, :], bd, ident)
                    bdt = consts.tile([C, C], BF16, tag=f"bdt{pair}")
                    nc.vector.tensor_copy(bdt, tb[:, 0, :])
                    mtp = qkvp.tile([C, QKV], F32, tag="qkv")
                    nc.tensor.matmul(
                        mtp[:, 0:128], lhsT=bdt, rhs=wo_s[:, pair, :],
                        start=True, stop=True, skip_group_check=True,
                    )
                    nc.vector.tensor_copy(mt, mtp[:, 0:128])

            # ---- pass 2: out = MT^T @ eqnA + b ----
            with tc.tile_pool(name="finp", bufs=4, space="PSUM") as finp:
                for blk in range(NBLK):
                    nsl = slice(blk * NB, (blk + 1) * NB)
                    fin = finp.tile([C, NB], F32, tag="fin")
                    nc.tensor.matmul(
                        fin, lhsT=MT01, rhs=eqnA[:, 0, nsl],
                        start=True, stop=False, skip_group_check=True,
                    )
                    nc.tensor.matmul(
                        fin, lhsT=MT23, rhs=eqnA[:, 1, nsl],
                        start=False, stop=True, skip_group_check=True,
                    )
                    osb = work.tile([C, NB], BF16, tag="osb")
                    if blk % 2 == 0:
                        nc.scalar.activation(
                            osb, fin, AF.Identity, bias=bo_s[:, 0:1], scale=1.0
                        )
                    else:
                        nc.vector.tensor_scalar_add(osb, fin, bo_s[:, 0:1])
                    nc.sync.dma_start(out=out[:, nsl], in_=osb)

    nc.compile()
    return nc


_NC_CACHE = None


def kernel(x, w_qkv, w_out, b_out):
    global _NC_CACHE
    if _NC_CACHE is None:
        _NC_CACHE = build_nc()
    nc = _NC_CACHE

    b = x.shape[0]
    bf = ml_dtypes.bfloat16
    wqT = np.ascontiguousarray(np.asarray(w_qkv, dtype=np.float32).T.astype(bf))
    woT = np.ascontiguousarray(np.asarray(w_out, dtype=np.float32).T.astype(bf))
    bo = np.ascontiguousarray(np.asarray(b_out, dtype=np.float32).reshape(C, 1))
    xb = np.asarray(x, dtype=np.float32).reshape(b, C, N).astype(bf)
    in_maps = [
        {"x": np.ascontiguousarray(xb[i]), "wqT": wqT, "woT": woT, "bo": bo}
        for i in range(b)
    ]
    res = run_bass_kernel_spmd(nc, in_maps, core_ids=list(range(b)))
    return np.stack(
        [
            res.results[i]["out"].astype(np.float32).reshape(C, 128, 128)
            for i in range(b)
        ]
    )


# revision 27
# speedup vs baseline: 1.1838x; 1.1838x over previous
"""LinearAttention Trainium2 Bass kernel.

kernel(**inputs) takes the full unsharded inputs from setup_inputs() and
returns the full output. Shards data-parallel over batch (b=8) across 8
NeuronCores; each core computes one batch item:

  qkv = w_qkv @ x            (layout B on chip: [n, 768], n on partitions)
  q = softmax_d(q); k = softmax_n(k)
  ctx[h] = ek_h^T @ v_h      (accumulated over n in PSUM; per-pair rhs is
                              [v_pair | ones] so each ctx matmul streams only
                              129 columns; the ones column yields sum_n ek)
  M^T = blockdiag(ctx/s_k)^T @ w_out^T   (folded once between passes)
  out = M @ eqnA + b_out     (eqnA = PE-transposed normalized exp(q))

The pass-1 loop is software-pipelined: ctx matmuls lag the producing
iteration by 1 and the q transposes / eqnA copies by LAG so the tensor
queue never waits on the scalar/vector/gpsimd chains.

Matmul operands are bf16; accumulation stays fp32 in PSUM. Output is
stored bf16 and widened to fp32 on the host.
"""

import numpy as np
import ml_dtypes

import concourse.bass as bass
import concourse.tile as tile
from concourse import bacc, mybir
from concourse.bass_utils import run_bass_kernel_spmd
from concourse.masks import make_identity

F32 = mybir.dt.float32
BF16 = mybir.dt.bfloat16
AF = mybir.ActivationFunctionType
X_AXIS = mybir.AxisListType.X

C = 128
N = 16384
HEADS = 4
DH = 64
INNER = HEADS * DH          # 256
QKV = 3 * INNER             # 768
NB = 512
SUB = NB // 128
NBLK = N // NB              # 32
NSUB = N // 128             # 128
LAG = 3                     # transpose/eqnA-copy skew (iterations)


def build_nc():
    nc = bacc.Bacc("TRN2", target_bir_lowering=False, debug=False, num_devices=8)

    x = nc.dram_tensor("x", [C, N], BF16, kind="ExternalInput")
    wqT = nc.dram_tensor("wqT", [C, QKV], BF16, kind="ExternalInput")
    woT = nc.dram_tensor("woT", [INNER, C], BF16, kind="ExternalInput")
    bo = nc.dram_tensor("bo", [C, 1], F32, kind="ExternalInput")
    out = nc.dram_tensor("out", [C, N], BF16, kind="ExternalOutput")

    with tile.TileContext(nc) as tc:
        with (
            tc.tile_pool(name="consts", bufs=1) as consts,
            tc.tile_pool(name="eqa", bufs=1) as eqa,
            tc.tile_pool(name="xin", bufs=4) as xin,
            tc.tile_pool(name="work", bufs=5) as work,
            tc.tile_pool(name="eqn", bufs=6) as eqnp,
            tc.tile_pool(name="vtp", bufs=4) as vtp,
            tc.tile_pool(name="small", bufs=4) as small,
        ):
            wq_s = consts.tile([C, QKV], BF16)
            nc.sync.dma_start(out=wq_s, in_=wqT[:, :])
            wo_s = consts.tile([C, 2, C], BF16)
            nc.sync.dma_start(out=wo_s[:, 0, :], in_=woT[0:128, :])
            nc.sync.dma_start(out=wo_s[:, 1, :], in_=woT[128:256, :])
            bo_s = consts.tile([C, 1], F32)
            nc.sync.dma_start(out=bo_s, in_=bo[:, :])
            ident = consts.tile([C, C], BF16)
            make_identity(nc, ident)

            # layout-A normalized exp(q): [:, 0, :] = heads 0/1, [:, 1, :] = 2/3
            eqnA = eqa.tile([C, 2, N], BF16)
            MT01 = consts.tile([C, C], BF16)
            MT23 = consts.tile([C, C], BF16)
            # blockdiag scratch: zeroed once here, only diag blocks written
            # at fold time
            bd01 = consts.tile([C, C], BF16)
            bd23 = consts.tile([C, C], BF16)
            nc.gpsimd.memset(bd01, 0.0)
            nc.gpsimd.memset(bd23, 0.0)

            with (
                tc.tile_pool(name="qkvp", bufs=3, space="PSUM") as qkvp,
                tc.tile_pool(name="trp", bufs=1, space="PSUM") as trp,
                tc.tile_pool(name="ctxp", bufs=1, space="PSUM") as ctxp,
            ):
                # both ctx accumulators in one PSUM bank: [:, 0, :] = heads
                # 0/1, [:, 1, :] = heads 2/3; col 128 = sum_n ek (ones trick).
                # Stride 160 keeps the second matmul's output 64B-aligned.
                ctx_pad = ctxp.tile([C, 2, 160], F32)
                ctx = ctx_pad[:, :, 0:129]
                # 3 transpose slots share one PSUM bank (512B each),
                # manually rotated; writers tensor-only, readers vector-only
                tr_all = trp.tile([C, 3, 2, C], BF16)

                x_blk = None
                qkv_t = [None] * NSUB   # PSUM qkv tiles
                eqk_t = [None] * NSUB   # SBUF exp(q|k)
                vt_t = [None] * NSUB    # SBUF [v01|ones|v23|ones]
                eqn_t = [None] * NSUB   # SBUF normalized exp(q)
                tr_t = [None] * NSUB    # PSUM transposed eqn

                for it in range(NSUB + LAG):
                    t = it if it < NSUB else None
                    if t is not None:
                        blk, s = divmod(t, SUB)
                        if s == 0:
                            x_blk = xin.tile([C, NB], BF16, tag="x_blk")
                            nc.sync.dma_start(
                                out=x_blk, in_=x[:, blk * NB : (blk + 1) * NB]
                            )
                        xs = x_blk[:, s * 128 : (s + 1) * 128]

                        qkv = qkvp.tile([C, QKV], F32, tag="qkv")
                        qkv_t[t] = qkv
                        nc.tensor.matmul(
                            qkv[:, 0:512], lhsT=xs, rhs=wq_s[:, 0:512],
                            start=True, stop=True, skip_group_check=True,
                        )
                        nc.tensor.matmul(
                            qkv[:, 512:768], lhsT=xs, rhs=wq_s[:, 512:768],
                            start=True, stop=True, skip_group_check=True,
                        )

                    # skewed tensor work: transposes for t-LAG
                    tp = it - LAG
                    if 0 <= tp < NSUB:
                        tr = tr_all[:, tp % 3]
                        tr_t[tp] = tr
                        eqns = eqn_t[tp]
                        nc.tensor.transpose(tr[:, 0, :], eqns[:, 0:2, :], ident)
                        nc.tensor.transpose(tr[:, 1, :], eqns[:, 2:4, :], ident)
                        eqn_t[tp] = None

                    # skewed ctx accumulation for t-2: its eqk/vt
                    # producers retired >1 iteration ago, so the tensor
                    # queue never stalls on the scalar copy gate
                    tc_ = it - 2
                    if 0 <= tc_ < NSUB:
                        eqkc = eqk_t[tc_]
                        vtc = vt_t[tc_]
                        nc.tensor.matmul(
                            ctx[:, 0, :], lhsT=eqkc[:, 4:6, :], rhs=vtc[:, 0, :],
                            start=(tc_ == 0), stop=(tc_ == NSUB - 1),
                            skip_group_check=True,
                        )
                        nc.tensor.matmul(
                            ctx[:, 1, :], lhsT=eqkc[:, 6:8, :], rhs=vtc[:, 1, :],
                            start=(tc_ == 0), stop=(tc_ == NSUB - 1),
                            skip_group_check=True,
                        )
                        if tc_ >= 2:
                            eqk_t[tc_ - 1] = None
                            vt_t[tc_ - 1] = None

                    if t is not None:
                        # one exp over q|k halves; heads 0..3 = q, 4..7 = k
                        eqk = work.tile([C, 8, DH], BF16, tag="eqk")
                        eqk_t[t] = eqk
                        nc.scalar.activation(eqk[:, :, :], qkv[:, 0:512], AF.Exp)

                        # v into SBUF as [v01 | ones | v23 | ones]
                        vt = vtp.tile([C, 2, 129], BF16, tag="vt")
                        vt_t[t] = vt
                        nc.scalar.copy(vt[:, :, 0:128], qkv[:, 512:768])
                        nc.gpsimd.memset(vt[:, :, 128:129], 1.0)

                        sq = small.tile([C, HEADS, 1], F32, tag="sq")
                        nc.vector.reduce_sum(sq, eqk[:, 0:4, :], axis=X_AXIS)
                        rq = small.tile([C, HEADS, 1], F32, tag="rq")
                        nc.vector.reciprocal(rq, sq)
                        eqn = eqnp.tile([C, HEADS, DH], BF16, tag="eqn")
                        eqn_t[t] = eqn
                        nc.gpsimd.tensor_mul(
                            eqn, eqk[:, 0:4, :], rq.broadcast_to([C, HEADS, DH])
                        )

                    # skewed eqnA copy for t-LAG (after its transposes above)
                    if 0 <= tp < NSUB:
                        nc.vector.tensor_copy(
                            eqnA[:, :, tp * 128 : (tp + 1) * 128], tr_t[tp]
                        )
                        tr_t[tp] = None

                # ---- fold: MT = (blockdiag(ctx/s_k))^T @ w_out^T ----
                r01 = small.tile([C, 1], F32, tag="r01")
                r23 = small.tile([C, 1], F32, tag="r23")
                nc.vector.reciprocal(r01, ctx[:, 0, 128:129])
                nc.vector.reciprocal(r23, ctx[:, 1, 128:129])
                nc.vector.tensor_scalar_mul(
                    bd01[0:64, 0:64], ctx[0:64, 0, 0:64], r01[0:64, 0:1]
                )
                nc.vector.tensor_scalar_mul(
                    bd01[64:128, 64:128], ctx[64:128, 0, 64:128], r01[64:128, 0:1]
                )
                nc.vector.tensor_scalar_mul(
                    bd23[0:64, 0:64], ctx[0:64, 1, 0:64], r23[0:64, 0:1]
                )
                nc.vector.tensor_scalar_mul(
                    bd23[64:128, 64:128], ctx[64:128, 1, 64:128], r23[64:128, 0:1]
                )

                for pair, bd, mt in ((0, bd01, MT01), (1, bd23, MT23)):
                    tb = tr_all[:, pair]
                    nc.tensor.transpose(tb[:, 0, :], bd, ident)
                    bdt = consts.tile([C, C], BF16, tag=f"bdt{pair}")
                    nc.vector.tensor_copy(bdt, tb[:, 0, :])
                    mtp = qkvp.tile([C, QKV], F32, tag="qkv")
                    nc.tensor.matmul(
                        mtp[:, 0:128], lhsT=bdt, rhs=wo_s[:, pair, :],
                        start=True, stop=True, skip_group_check=True,
                    )
                    nc.vector.tensor_copy(mt, mtp[:, 0:128])

            # ---- pass 2: out = MT^T @ eqnA + b ----
            with tc.tile_pool(name="finp", bufs=4, space="PSUM") as finp:
                for blk in range(NBLK):
                    nsl = slice(blk * NB, (blk + 1) * NB)
                    fin = finp.tile([C, NB], F32, tag="fin")
                    nc.tensor.matmul(
                        fin, lhsT=MT01, rhs=eqnA[:, 0, nsl],
                        start=True, stop=False, skip_group_check=True,
                    )
                    nc.tensor.matmul(
                        fin, lhsT=MT23, rhs=eqnA[:, 1, nsl],
                        start=False, stop=True, skip_group_check=True,
                    )
                    osb = work.tile([C, NB], BF16, tag="osb")
                    if blk % 2 == 0:
                        nc.scalar.activation(
                            osb, fin, AF.Identity, bias=bo_s[:, 0:1], scale=1.0
                        )
                    else:
                        nc.vector.tensor_scalar_add(osb, fin, bo_s[:, 0:1])
                    nc.sync.dma_start(out=out[:, nsl], in_=osb)

    nc.compile()
    return nc


_NC_CACHE = None


def kernel(x, w_qkv, w_out, b_out):
    global _NC_CACHE
    if _NC_CACHE is None:
        _NC_CACHE = build_nc()
    nc = _NC_CACHE

    b = x.shape[0]
    bf = ml_dtypes.bfloat16
    wqT = np.ascontiguousarray(np.asarray(w_qkv, dtype=np.float32).T.astype(bf))
    woT = np.ascontiguousarray(np.asarray(w_out, dtype=np.float32).T.astype(bf))
    bo = np.ascontiguousarray(np.asarray(b_out, dtype=np.float32).reshape(C, 1))
    xb = np.asarray(x, dtype=np.float32).reshape(b, C, N).astype(bf)
    in_maps = [
        {"x": np.ascontiguousarray(xb[i]), "wqT": wqT, "woT": woT, "bo": bo}
        for i in range(b)
    ]
    res = run_bass_kernel_spmd(nc, in_maps, core_ids=list(range(b)))
    return np.stack(
        [
            res.results[i]["out"].astype(np.float32).reshape(C, 128, 128)
            for i in range(b)
        ]
    )


# revision 28
# speedup vs baseline: 1.3882x; 1.1727x over previous
"""LinearAttention Trainium2 Bass kernel.

kernel(**inputs) takes the full unsharded inputs from setup_inputs() and
returns the full output. Shards data-parallel over batch (b=8) across 8
NeuronCores; each core computes one batch item:

  qkv = w_qkv @ x            (layout B on chip: [n, 768], n on partitions)
  q = softmax_d(q); k = softmax_n(k)
  ctx[h] = ek_h^T @ v_h      (accumulated over n in PSUM; per-pair rhs is
                              [v_pair | ones] so each ctx matmul streams only
                              129 columns; the ones column yields sum_n ek)
  M^T = blockdiag(ctx/s_k)^T @ w_out^T   (folded once between passes)
  out = M @ eqnA + b_out     (eqnA = PE-transposed normalized exp(q))

The pass-1 loop is software-pipelined: ctx matmuls lag the producing
iteration by 1 and the q transposes / eqnA copies by LAG so the tensor
queue never waits on the scalar/vector/gpsimd chains.

Matmul operands are bf16; accumulation stays fp32 in PSUM. Output is
stored bf16 and widened to fp32 on the host.
"""

import numpy as np
import ml_dtypes

import concourse.bass as bass
import concourse.tile as tile
from concourse import bacc, mybir
from concourse.bass_utils import run_bass_kernel_spmd
from concourse.masks import make_identity

F32 = mybir.dt.float32
BF16 = mybir.dt.bfloat16
AF = mybir.ActivationFunctionType
X_AXIS = mybir.AxisListType.X

C = 128
N = 16384
HEADS = 4
DH = 64
INNER = HEADS * DH          # 256
QKV = 3 * INNER             # 768
NB = 512
SUB = NB // 128
NBLK = N // NB              # 32
NSUB = N // 128             # 128
LAG = 3                     # transpose/eqnA-copy skew (iterations)


def build_nc():
    nc = bacc.Bacc("TRN2", target_bir_lowering=False, debug=False, num_devices=8)

    x = nc.dram_tensor("x", [C, N], BF16, kind="ExternalInput")
    wqT = nc.dram_tensor("wqT", [C, QKV], BF16, kind="ExternalInput")
    woT = nc.dram_tensor("woT", [INNER, C], BF16, kind="ExternalInput")
    bo = nc.dram_tensor("bo", [C, 1], F32, kind="ExternalInput")
    out = nc.dram_tensor("out", [C, N], BF16, kind="ExternalOutput")

    with tile.TileContext(nc) as tc:
        with (
            tc.tile_pool(name="consts", bufs=1) as consts,
            tc.tile_pool(name="eqa", bufs=1) as eqa,
            tc.tile_pool(name="xin", bufs=4) as xin,
            tc.tile_pool(name="work", bufs=5) as work,
            tc.tile_pool(name="eqn", bufs=6) as eqnp,
            tc.tile_pool(name="vtp", bufs=4) as vtp,
            tc.tile_pool(name="small", bufs=4) as small,
        ):
            wq_s = consts.tile([C, QKV], BF16)
            nc.sync.dma_start(out=wq_s, in_=wqT[:, :])
            wo_s = consts.tile([C, 2, C], BF16)
            nc.sync.dma_start(out=wo_s[:, 0, :], in_=woT[0:128, :])
            nc.sync.dma_start(out=wo_s[:, 1, :], in_=woT[128:256, :])
            bo_s = consts.tile([C, 1], F32)
            nc.sync.dma_start(out=bo_s, in_=bo[:, :])
            ident = consts.tile([C, C], BF16)
            make_identity(nc, ident)
            # pre-fire the scalar engine's exp table load here (overlapped
            # with weight DMAs) instead of serially at the first real exp
            warm = consts.tile([C, 1], F32)
            nc.gpsimd.memset(warm, 0.0)
            wexp = consts.tile([C, 1], BF16)
            nc.scalar.activation(wexp, warm, AF.Exp)

            # layout-A normalized exp(q): [:, 0, :] = heads 0/1, [:, 1, :] = 2/3
            eqnA = eqa.tile([C, 2, N], BF16)
            MT01 = consts.tile([C, C], BF16)
            MT23 = consts.tile([C, C], BF16)
            # blockdiag scratch: zeroed once here, only diag blocks written
            # at fold time
            bd01 = consts.tile([C, C], BF16)
            bd23 = consts.tile([C, C], BF16)
            nc.gpsimd.memset(bd01, 0.0)
            nc.gpsimd.memset(bd23, 0.0)

            with (
                tc.tile_pool(name="qkvp", bufs=3, space="PSUM") as qkvp,
                tc.tile_pool(name="trp", bufs=1, space="PSUM") as trp,
                tc.tile_pool(name="ctxp", bufs=1, space="PSUM") as ctxp,
            ):
                # both ctx accumulators in one PSUM bank: [:, 0, :] = heads
                # 0/1, [:, 1, :] = heads 2/3; col 128 = sum_n ek (ones trick).
                # Stride 160 keeps the second matmul's output 64B-aligned.
                ctx_pad = ctxp.tile([C, 2, 160], F32)
                ctx = ctx_pad[:, :, 0:129]
                # 3 transpose slots share one PSUM bank (512B each),
                # manually rotated; writers tensor-only, readers vector-only
                tr_all = trp.tile([C, 3, 2, C], BF16)

                x_blk = None
                qkv_t = [None] * NSUB   # PSUM qkv tiles
                eqk_t = [None] * NSUB   # SBUF exp(q|k)
                vt_t = [None] * NSUB    # SBUF [v01|ones|v23|ones]
                eqn_t = [None] * NSUB   # SBUF normalized exp(q)
                tr_t = [None] * NSUB    # PSUM transposed eqn

                for it in range(NSUB + LAG):
                    t = it if it < NSUB else None
                    if t is not None:
                        blk, s = divmod(t, SUB)
                        if s == 0:
                            x_blk = xin.tile([C, NB], BF16, tag="x_blk")
                            nc.sync.dma_start(
                                out=x_blk, in_=x[:, blk * NB : (blk + 1) * NB]
                            )
                        xs = x_blk[:, s * 128 : (s + 1) * 128]

                        qkv = qkvp.tile([C, QKV], F32, tag="qkv")
                        qkv_t[t] = qkv
                        nc.tensor.matmul(
                            qkv[:, 0:512], lhsT=xs, rhs=wq_s[:, 0:512],
                            start=True, stop=True, skip_group_check=True,
                        )
                        nc.tensor.matmul(
                            qkv[:, 512:768], lhsT=xs, rhs=wq_s[:, 512:768],
                            start=True, stop=True, skip_group_check=True,
                        )

                    # skewed tensor work: transposes for t-LAG
                    tp = it - LAG
                    if 0 <= tp < NSUB:
                        tr = tr_all[:, tp % 3]
                        tr_t[tp] = tr
                        eqns = eqn_t[tp]
                        nc.tensor.transpose(tr[:, 0, :], eqns[:, 0:2, :], ident)
                        nc.tensor.transpose(tr[:, 1, :], eqns[:, 2:4, :], ident)
                        eqn_t[tp] = None

                    # skewed ctx accumulation for t-2: its eqk/vt
                    # producers retired >1 iteration ago, so the tensor
                    # queue never stalls on the scalar copy gate
                    tc_ = it - 2
                    if 0 <= tc_ < NSUB:
                        eqkc = eqk_t[tc_]
                        vtc = vt_t[tc_]
                        nc.tensor.matmul(
                            ctx[:, 0, :], lhsT=eqkc[:, 4:6, :], rhs=vtc[:, 0, :],
                            start=(tc_ == 0), stop=(tc_ == NSUB - 1),
                            skip_group_check=True,
                        )
                        nc.tensor.matmul(
                            ctx[:, 1, :], lhsT=eqkc[:, 6:8, :], rhs=vtc[:, 1, :],
                            start=(tc_ == 0), stop=(tc_ == NSUB - 1),
                            skip_group_check=True,
                        )
                        if tc_ >= 2:
                            eqk_t[tc_ - 1] = None
                            vt_t[tc_ - 1] = None

                    if t is not None:
                        # one exp over q|k halves; heads 0..3 = q, 4..7 = k
                        eqk = work.tile([C, 8, DH], BF16, tag="eqk")
                        eqk_t[t] = eqk
                        nc.scalar.activation(eqk[:, :, :], qkv[:, 0:512], AF.Exp)

                        # v into SBUF as [v01 | ones | v23 | ones]
                        vt = vtp.tile([C, 2, 129], BF16, tag="vt")
                        vt_t[t] = vt
                        nc.scalar.copy(vt[:, :, 0:128], qkv[:, 512:768])
                        nc.gpsimd.memset(vt[:, :, 128:129], 1.0)

                        sq = small.tile([C, HEADS, 1], F32, tag="sq")
                        nc.vector.reduce_sum(sq, eqk[:, 0:4, :], axis=X_AXIS)
                        rq = small.tile([C, HEADS, 1], F32, tag="rq")
                        nc.vector.reciprocal(rq, sq)
                        eqn = eqnp.tile([C, HEADS, DH], BF16, tag="eqn")
                        eqn_t[t] = eqn
                        nc.gpsimd.tensor_mul(
                            eqn, eqk[:, 0:4, :], rq.broadcast_to([C, HEADS, DH])
                        )

                    # skewed eqnA copy for t-LAG (after its transposes above)
                    if 0 <= tp < NSUB:
                        nc.vector.tensor_copy(
                            eqnA[:, :, tp * 128 : (tp + 1) * 128], tr_t[tp]
                        )
                        tr_t[tp] = None

                # ---- fold: MT = (blockdiag(ctx/s_k))^T @ w_out^T ----
                r01 = small.tile([C, 1], F32, tag="r01")
                r23 = small.tile([C, 1], F32, tag="r23")
                nc.vector.reciprocal(r01, ctx[:, 0, 128:129])
                nc.vector.reciprocal(r23, ctx[:, 1, 128:129])
                nc.vector.tensor_scalar_mul(
                    bd01[0:64, 0:64], ctx[0:64, 0, 0:64], r01[0:64, 0:1]
                )
                nc.vector.tensor_scalar_mul(
                    bd01[64:128, 64:128], ctx[64:128, 0, 64:128], r01[64:128, 0:1]
                )
                nc.vector.tensor_scalar_mul(
                    bd23[0:64, 0:64], ctx[0:64, 1, 0:64], r23[0:64, 0:1]
                )
                nc.vector.tensor_scalar_mul(
                    bd23[64:128, 64:128], ctx[64:128, 1, 64:128], r23[64:128, 0:1]
                )

                for pair, bd, mt in ((0, bd01, MT01), (1, bd23, MT23)):
                    tb = tr_all[:, pair]
                    nc.tensor.transpose(tb[:, 0, :], bd, ident)
                    bdt = consts.tile([C, C], BF16, tag=f"bdt{pair}")
                    nc.vector.tensor_copy(bdt, tb[:, 0, :])
                    mtp = qkvp.tile([C, QKV], F32, tag="qkv")
                    nc.tensor.matmul(
                        mtp[:, 0:128], lhsT=bdt, rhs=wo_s[:, pair, :],
                        start=True, stop=True, skip_group_check=True,
                    )
                    nc.vector.tensor_copy(mt, mtp[:, 0:128])

            # ---- pass 2: out = MT^T @ eqnA + b ----
            with tc.tile_pool(name="finp", bufs=4, space="PSUM") as finp:
                for blk in range(NBLK):
                    nsl = slice(blk * NB, (blk + 1) * NB)
                    fin = finp.tile([C, NB], F32, tag="fin")
                    nc.tensor.matmul(
                        fin, lhsT=MT01, rhs=eqnA[:, 0, nsl],
                        start=True, stop=False, skip_group_check=True,
                    )
                    nc.tensor.matmul(
                        fin, lhsT=MT23, rhs=eqnA[:, 1, nsl],
                        start=False, stop=True, skip_group_check=True,
                    )
                    osb = work.tile([C, NB], BF16, tag="osb")
                    if blk % 2 == 0:
                        nc.scalar.activation(
                            osb, fin, AF.Identity, bias=bo_s[:, 0:1], scale=1.0
                        )
                    else:
                        nc.vector.tensor_scalar_add(osb, fin, bo_s[:, 0:1])
                    nc.sync.dma_start(out=out[:, nsl], in_=osb)

    nc.compile()
    return nc


_NC_CACHE = None


def kernel(x, w_qkv, w_out, b_out):
    global _NC_CACHE
    if _NC_CACHE is None:
        _NC_CACHE = build_nc()
    nc = _NC_CACHE

    b = x.shape[0]
    bf = ml_dtypes.bfloat16
    wqT = np.ascontiguousarray(np.asarray(w_qkv, dtype=np.float32).T.astype(bf))
    woT = np.ascontiguousarray(np.asarray(w_out, dtype=np.float32).T.astype(bf))
    bo = np.ascontiguousarray(np.asarray(b_out, dtype=np.float32).reshape(C, 1))
    xb = np.asarray(x, dtype=np.float32).reshape(b, C, N).astype(bf)
    in_maps = [
        {"x": np.ascontiguousarray(xb[i]), "wqT": wqT, "woT": woT, "bo": bo}
        for i in range(b)
    ]
    res = run_bass_kernel_spmd(nc, in_maps, core_ids=list(range(b)))
    return np.stack(
        [
            res.results[i]["out"].astype(np.float32).reshape(C, 128, 128)
            for i in range(b)
        ]
    )


# revision 29
# speedup vs baseline: 1.5861x; 1.1426x over previous
"""LinearAttention Trainium2 Bass kernel.

kernel(**inputs) takes the full unsharded inputs from setup_inputs() and
returns the full output. Shards data-parallel over batch (b=8) across 8
NeuronCores; each core computes one batch item:

  qkv = w_qkv @ x            (layout B on chip: [n, 768], n on partitions)
  q = softmax_d(q); k = softmax_n(k)
  ctx[h] = ek_h^T @ v_h      (accumulated over n in PSUM; per-pair rhs is
                              [v_pair | ones] so each ctx matmul streams only
                              129 columns; the ones column yields sum_n ek)
  M^T = blockdiag(ctx/s_k)^T @ w_out^T   (folded once between passes)
  out = M @ eqnA + b_out     (eqnA = PE-transposed normalized exp(q))

The pass-1 loop is software-pipelined: ctx matmuls lag the producing
iteration by 1 and the q transposes / eqnA copies by LAG so the tensor
queue never waits on the scalar/vector/gpsimd chains.

Matmul operands are bf16; accumulation stays fp32 in PSUM. Output is
stored bf16 and widened to fp32 on the host.
"""

import numpy as np
import ml_dtypes

import concourse.bass as bass
import concourse.tile as tile
from concourse import bacc, mybir
from concourse.bass_utils import run_bass_kernel_spmd
from concourse.masks import make_identity

F32 = mybir.dt.float32
BF16 = mybir.dt.bfloat16
AF = mybir.ActivationFunctionType
X_AXIS = mybir.AxisListType.X

C = 128
N = 16384
HEADS = 4
DH = 64
INNER = HEADS * DH          # 256
QKV = 3 * INNER             # 768
NB = 512
SUB = NB // 128
NBLK = N // NB              # 32
NSUB = N // 128             # 128
LAG = 3                     # transpose/eqnA-copy skew (iterations)


def build_nc():
    nc = bacc.Bacc("TRN2", target_bir_lowering=False, debug=False, num_devices=8)

    x = nc.dram_tensor("x", [C, N], BF16, kind="ExternalInput")
    wqT = nc.dram_tensor("wqT", [C, QKV], BF16, kind="ExternalInput")
    woT = nc.dram_tensor("woT", [INNER, C], BF16, kind="ExternalInput")
    bo = nc.dram_tensor("bo", [C, 1], F32, kind="ExternalInput")
    out = nc.dram_tensor("out", [C, N], BF16, kind="ExternalOutput")

    with tile.TileContext(nc) as tc:
        with (
            tc.tile_pool(name="consts", bufs=1) as consts,
            tc.tile_pool(name="eqa", bufs=1) as eqa,
            tc.tile_pool(name="xin", bufs=4) as xin,
            tc.tile_pool(name="work", bufs=5) as work,
            tc.tile_pool(name="eqn", bufs=6) as eqnp,
            tc.tile_pool(name="vtp", bufs=4) as vtp,
            tc.tile_pool(name="small", bufs=4) as small,
        ):
            wq_s = consts.tile([C, QKV], BF16)
            nc.sync.dma_start(out=wq_s, in_=wqT[:, :])
            wo_s = consts.tile([C, 2, C], BF16)
            nc.sync.dma_start(out=wo_s[:, 0, :], in_=woT[0:128, :])
            nc.sync.dma_start(out=wo_s[:, 1, :], in_=woT[128:256, :])
            bo_s = consts.tile([C, 1], F32)
            nc.sync.dma_start(out=bo_s, in_=bo[:, :])
            ident = consts.tile([C, C], BF16)
            make_identity(nc, ident)
            # pre-fire the scalar engine's exp table load here (overlapped
            # with weight DMAs) instead of serially at the first real exp
            warm = consts.tile([C, 1], F32)
            nc.gpsimd.memset(warm, 0.0)
            wexp = consts.tile([C, 1], BF16)
            nc.scalar.activation(wexp, warm, AF.Exp)

            # layout-A normalized exp(q): [:, 0, :] = heads 0/1, [:, 1, :] = 2/3
            eqnA = eqa.tile([C, 2, N], BF16)
            MT01 = consts.tile([C, C], BF16)
            MT23 = consts.tile([C, C], BF16)
            # blockdiag scratch: zeroed once here, only diag blocks written
            # at fold time
            bd01 = consts.tile([C, C], BF16)
            bd23 = consts.tile([C, C], BF16)
            nc.gpsimd.memset(bd01, 0.0)
            nc.gpsimd.memset(bd23, 0.0)

            with (
                tc.tile_pool(name="qkvp", bufs=3, space="PSUM") as qkvp,
                tc.tile_pool(name="trp", bufs=1, space="PSUM") as trp,
                tc.tile_pool(name="ctxp", bufs=1, space="PSUM") as ctxp,
            ):
                # both ctx accumulators in one PSUM bank: [:, 0, :] = heads
                # 0/1, [:, 1, :] = heads 2/3; col 128 = sum_n ek (ones trick).
                # Stride 160 keeps the second matmul's output 64B-aligned.
                ctx_pad = ctxp.tile([C, 2, 160], F32)
                ctx = ctx_pad[:, :, 0:129]
                # 3 transpose slots share one PSUM bank (512B each),
                # manually rotated; writers tensor-only, readers vector-only
                tr_all = trp.tile([C, 3, 2, C], BF16)

                x_blk = None
                qkv_t = [None] * NSUB   # PSUM qkv tiles
                eqk_t = [None] * NSUB   # SBUF exp(q|k)
                vt_t = [None] * NSUB    # SBUF [v01|ones|v23|ones]
                eqn_t = [None] * NSUB   # SBUF normalized exp(q)
                tr_t = [None] * NSUB    # PSUM transposed eqn

                for it in range(NSUB + LAG):
                    t = it if it < NSUB else None
                    if t is not None:
                        blk, s = divmod(t, SUB)
                        if s == 0:
                            x_blk = xin.tile([C, NB], BF16, tag="x_blk")
                            nc.sync.dma_start(
                                out=x_blk, in_=x[:, blk * NB : (blk + 1) * NB]
                            )
                        xs = x_blk[:, s * 128 : (s + 1) * 128]

                        qkv = qkvp.tile([C, QKV], F32, tag="qkv")
                        qkv_t[t] = qkv
                        nc.tensor.matmul(
                            qkv[:, 0:512], lhsT=xs, rhs=wq_s[:, 0:512],
                            start=True, stop=True, skip_group_check=True,
                        )
                        nc.tensor.matmul(
                            qkv[:, 512:768], lhsT=xs, rhs=wq_s[:, 512:768],
                            start=True, stop=True, skip_group_check=True,
                        )

                    # skewed tensor work: transposes for t-LAG
                    tp = it - LAG
                    if 0 <= tp < NSUB:
                        tr = tr_all[:, tp % 3]
                        tr_t[tp] = tr
                        eqns = eqn_t[tp]
                        nc.tensor.transpose(tr[:, 0, :], eqns[:, 0:2, :], ident)
                        nc.tensor.transpose(tr[:, 1, :], eqns[:, 2:4, :], ident)
                        eqn_t[tp] = None

                    # skewed ctx accumulation for t-2: its eqk/vt
                    # producers retired >1 iteration ago, so the tensor
                    # queue never stalls on the scalar copy gate
                    tc_ = it - 2
                    if 0 <= tc_ < NSUB:
                        eqkc = eqk_t[tc_]
                        vtc = vt_t[tc_]
                        nc.tensor.matmul(
                            ctx[:, 0, :], lhsT=eqkc[:, 4:6, :], rhs=vtc[:, 0, :],
                            start=(tc_ == 0), stop=(tc_ == NSUB - 1),
                            skip_group_check=True,
                        )
                        nc.tensor.matmul(
                            ctx[:, 1, :], lhsT=eqkc[:, 6:8, :], rhs=vtc[:, 1, :],
                            start=(tc_ == 0), stop=(tc_ == NSUB - 1),
                            skip_group_check=True,
                        )
                        if tc_ >= 2:
                            eqk_t[tc_ - 1] = None
                            vt_t[tc_ - 1] = None

                    if t is not None:
                        # one exp over q|k halves; heads 0..3 = q, 4..7 = k
                        eqk = work.tile([C, 8, DH], BF16, tag="eqk")
                        eqk_t[t] = eqk
                        vt = vtp.tile([C, 2, 129], BF16, tag="vt")
                        vt_t[t] = vt
                        nc.scalar.activation(eqk[:, :, :], qkv[:, 0:512], AF.Exp)
                        nc.gpsimd.memset(vt[:, :, 128:129], 1.0)

                        sq = small.tile([C, HEADS, 1], F32, tag="sq")
                        nc.vector.reduce_sum(sq, eqk[:, 0:4, :], axis=X_AXIS)
                        rq = small.tile([C, HEADS, 1], F32, tag="rq")
                        nc.vector.reciprocal(rq, sq)
                        eqn = eqnp.tile([C, HEADS, DH], BF16, tag="eqn")
                        eqn_t[t] = eqn
                        nc.gpsimd.tensor_mul(
                            eqn, eqk[:, 0:4, :], rq.broadcast_to([C, HEADS, DH])
                        )

                    # eqnA copy on scalar: no consumer until pass 2, so it
                    # can sit second in the scalar queue with zero coupling
                    if 0 <= tp < NSUB:
                        nc.scalar.copy(
                            eqnA[:, :, tp * 128 : (tp + 1) * 128], tr_t[tp]
                        )
                        tr_t[tp] = None
                    # v copy on vector tail (single writer; qkv slot release
                    # has 2 iterations of slack at qkvp bufs=3)
                    if t is not None:
                        nc.vector.tensor_copy(vt[:, :, 0:128], qkv[:, 512:768])

                # ---- fold: MT = (blockdiag(ctx/s_k))^T @ w_out^T ----
                r01 = small.tile([C, 1], F32, tag="r01")
                r23 = small.tile([C, 1], F32, tag="r23")
                nc.vector.reciprocal(r01, ctx[:, 0, 128:129])
                nc.vector.reciprocal(r23, ctx[:, 1, 128:129])
                nc.vector.tensor_scalar_mul(
                    bd01[0:64, 0:64], ctx[0:64, 0, 0:64], r01[0:64, 0:1]
                )
                nc.vector.tensor_scalar_mul(
                    bd01[64:128, 64:128], ctx[64:128, 0, 64:128], r01[64:128, 0:1]
                )
                nc.vector.tensor_scalar_mul(
                    bd23[0:64, 0:64], ctx[0:64, 1, 0:64], r23[0:64, 0:1]
                )
                nc.vector.tensor_scalar_mul(
                    bd23[64:128, 64:128], ctx[64:128, 1, 64:128], r23[64:128, 0:1]
                )

                for pair, bd, mt in ((0, bd01, MT01), (1, bd23, MT23)):
                    tb = tr_all[:, pair]
                    nc.tensor.transpose(tb[:, 0, :], bd, ident)
                    bdt = consts.tile([C, C], BF16, tag=f"bdt{pair}")
                    nc.vector.tensor_copy(bdt, tb[:, 0, :])
                    mtp = qkvp.tile([C, QKV], F32, tag="qkv")
                    nc.tensor.matmul(
                        mtp[:, 0:128], lhsT=bdt, rhs=wo_s[:, pair, :],
                        start=True, stop=True, skip_group_check=True,
                    )
                    nc.vector.tensor_copy(mt, mtp[:, 0:128])

            # ---- pass 2: out = MT^T @ eqnA + b ----
            with tc.tile_pool(name="finp", bufs=4, space="PSUM") as finp:
                for blk in range(NBLK):
                    nsl = slice(blk * NB, (blk + 1) * NB)
                    fin = finp.tile([C, NB], F32, tag="fin")
                    nc.tensor.matmul(
                        fin, lhsT=MT01, rhs=eqnA[:, 0, nsl],
                        start=True, stop=False, skip_group_check=True,
                    )
                    nc.tensor.matmul(
                        fin, lhsT=MT23, rhs=eqnA[:, 1, nsl],
                        start=False, stop=True, skip_group_check=True,
                    )
                    osb = work.tile([C, NB], BF16, tag="osb")
                    if blk % 2 == 0:
                        nc.scalar.activation(
                            osb, fin, AF.Identity, bias=bo_s[:, 0:1], scale=1.0
                        )
                    else:
                        nc.vector.tensor_scalar_add(osb, fin, bo_s[:, 0:1])
                    nc.sync.dma_start(out=out[:, nsl], in_=osb)

    nc.compile()
    return nc


_NC_CACHE = None


def kernel(x, w_qkv, w_out, b_out):
    global _NC_CACHE
    if _NC_CACHE is None:
        _NC_CACHE = build_nc()
    nc = _NC_CACHE

    b = x.shape[0]
    bf = ml_dtypes.bfloat16
    wqT = np.ascontiguousarray(np.asarray(w_qkv, dtype=np.float32).T.astype(bf))
    woT = np.ascontiguousarray(np.asarray(w_out, dtype=np.float32).T.astype(bf))
    bo = np.ascontiguousarray(np.asarray(b_out, dtype=np.float32).reshape(C, 1))
    xb = np.asarray(x, dtype=np.float32).reshape(b, C, N).astype(bf)
    in_maps = [
        {"x": np.ascontiguousarray(xb[i]), "wqT": wqT, "woT": woT, "bo": bo}
        for i in range(b)
    ]
    res = run_bass_kernel_spmd(nc, in_maps, core_ids=list(range(b)))
    return np.stack(
        [
            res.results[i]["out"].astype(np.float32).reshape(C, 128, 128)
            for i in range(b)
        ]
    )
